# revision 31
# baseline (speedup 1.0000x reference)
"""Trainium2 Bass kernel for a dense transformer block (B=2, T=2048, C=1024, H=16).

Sharding: DP2 (batch -> core groups {0-3},{4-7}) x TP4 within a group:
  - attention: Megatron head-parallel (4 heads/core), row-parallel out-proj,
    pipelined ReduceScatter(add) over the group. Query chunks processed in
    DESCENDING order (3,2,1,0); chunk 3 heads 0-1 are interleaved INTO the
    feed loop (key blocks processed in descending order as their tiles
    arrive) so the first ReduceScatter fires right after the feed finishes.
  - MLP: sequence-parallel (each core computes its 512 rows with the FULL
    fc / proj weights). No other collective.

Row ownership: core at group position p owns rows {512j+128p .. +128} for
j in 0..3 (strip j comes from query chunk j).

Device layout notes:
  - Activations feeding matmuls are kept transposed [features, tokens].
  - LN affine params folded into following weights on host; q-scale folded
    into W_q/b_q; v-bias and out-proj bias folded into xres on the host.
  - Softmax: scores^T[k,q] psum tiles; exp on ScalarE (no max subtraction:
    scores ~N(0,1)); denominator via ones-column appended to V (row 64 of
    the PV output); 1/denom via DRAM-bounce partition-broadcast (DMA
    straight out of PSUM), reciprocal taken on the broadcast [64,512],
    normalization deferred one head.
  - wfc fully resident in SBUF (right-side stack, loads overlap attention);
    wmp streamed (twice, once per g half-pass).
  - Matmul operands fp16 (full PE rate); accumulation, softmax statistics,
    residuals and LN are fp32.
"""

import os
import sys

import numpy as np

for _p in ("/opt/trn_rl_repo", "/root/.axon_site/_ro/trn_rl_repo"):
    if os.path.isdir(_p) and _p not in sys.path:
        sys.path.insert(0, _p)

import concourse.bass as bass
import concourse.tile as tile
from concourse import bacc, mybir
from concourse.bass_utils import run_bass_kernel_spmd

B, T, C, H = 2, 2048, 1024, 16
D = C // H  # 64
EPS = 1e-5
N_CORES = 8
TP = 4            # tensor-parallel group size
HPC = 4           # heads per core
ROWS = T // TP    # 512 token rows owned per core
F32 = mybir.dt.float32
F16 = mybir.dt.float16  # matmul operand dtype

TT = T // 128     # 16 token tiles
CB = C // 128     # 8 channel blocks
QC = T // 512     # 4 query chunks / row blocks
RG = [[0, 1, 2, 3], [4, 5, 6, 7]]

GELU_NAME = "Gelu_apprx_tanh"


def _bc(ap, p):
    """Broadcast a DRAM AP across p partitions (prepend stride-0 dim)."""
    return bass.AP(tensor=ap.tensor, offset=ap.offset, ap=[[0, p], *ap.ap])


def build_program():
    nc = bacc.Bacc(
        "TRN2", target_bir_lowering=False, debug=False, num_devices=N_CORES
    )

    # ---- I/O ----
    x16_d = nc.dram_tensor("x16", [T, C], F16, kind="ExternalInput").ap()
    wqk_d = nc.dram_tensor("wqk", [C, 512], F16, kind="ExternalInput").ap()
    bqk_d = nc.dram_tensor("bqk", [512], F32, kind="ExternalInput").ap()
    wv_d = nc.dram_tensor("wv", [C, 256], F16, kind="ExternalInput").ap()
    wproj_d = nc.dram_tensor("wproj", [256, C], F16, kind="ExternalInput").ap()
    wfc_d = nc.dram_tensor("wfc", [C, 4 * C], F16, kind="ExternalInput").ap()
    bfc_d = nc.dram_tensor("bfc", [4 * C], F32, kind="ExternalInput").ap()
    wmp_d = nc.dram_tensor("wmp", [4 * C, C], F16, kind="ExternalInput").ap()
    bmp_d = nc.dram_tensor("bmp", [C], F32, kind="ExternalInput").ap()
    ident_d = nc.dram_tensor("ident", [128, 128], F16, kind="ExternalInput").ap()
    trim_d = nc.dram_tensor("trim", [128, 128], F16, kind="ExternalInput").ap()
    xres_d = nc.dram_tensor("xres", [ROWS, C], F32, kind="ExternalInput").ap()
    out_d = nc.dram_tensor("out", [ROWS, C], F32, kind="ExternalOutput").ap()

    with tile.TileContext(nc) as tc:
        _body(nc, tc, locals())
    nc.compile()
    return nc


def _body(nc, tc, io):
    x16_d = io["x16_d"]; wqk_d = io["wqk_d"]; bqk_d = io["bqk_d"]
    wv_d = io["wv_d"]; wproj_d = io["wproj_d"]; wfc_d = io["wfc_d"]
    bfc_d = io["bfc_d"]; wmp_d = io["wmp_d"]; bmp_d = io["bmp_d"]
    ident_d = io["ident_d"]; trim_d = io["trim_d"]; xres_d = io["xres_d"]
    out_d = io["out_d"]

    AF = mybir.ActivationFunctionType
    OP = mybir.AluOpType

    consts = tc.alloc_tile_pool(name="consts", bufs=1)
    dram = tc.alloc_tile_pool(name="dram", bufs=1, space="DRAM")

    # ---- PSUM pools (8 banks; phased: paux1+pyp2+psf5 / +sreg4 / +psm7) --
    paux = tc.alloc_tile_pool(name="paux", bufs=1, space="PSUM")  # proj/ln2T
    pyp = tc.alloc_tile_pool(name="pyp", bufs=2, space="PSUM")    # y^T accum
    psf = tc.alloc_tile_pool(name="psf", bufs=5, space="PSUM")    # feed+attn3

    # ---------- constants ----------
    epsb = consts.tile([128, 1], F32)
    nc.vector.memset(epsb, EPS)
    ones_c = consts.tile([128, HPC, 1], F16)
    nc.vector.memset(ones_c, 1.0)
    zrow = consts.tile([1, 512], F16)
    nc.vector.memset(zrow, 0.0)
    ident = consts.tile([128, 128], F16)
    nc.sync.dma_start(out=ident, in_=ident_d)
    trim = consts.tile([128, 128], F16)
    nc.sync.dma_start(out=trim, in_=trim_d)
    bqk_sb = consts.tile([128, 4], F32)
    nc.sync.dma_start(out=bqk_sb, in_=bqk_d.rearrange("(m p) -> p m", p=128))
    bfc_sb = consts.tile([128, 32], F32)
    nc.sync.dma_start(out=bfc_sb, in_=bfc_d.rearrange("(m p) -> p m", p=128))
    bmp_bc = consts.tile([128, C], F32)

    # DRAM scratch (fp16 collective payload; per-chunk tiles avoid WAR
    # serialization between one chunk's RS read and the next chunk's
    # proj writes)
    attn_part = [dram.tile([512, C], F16, tag=f"ap{j}", name=f"ap{j}")
                 for j in range(QC)]
    rs_out = [dram.tile([128, C], F16, tag=f"rs{j}", name=f"rs{j}")
              for j in range(QC)]
    dnrm = [dram.tile([HPC, 512], F32, tag=f"dn{j}", name=f"dn{j}")
            for j in range(QC)]
    dnrm2 = [dram.tile([HPC, 512], F32, tag=f"dn2{j}", name=f"dn2{j}")
             for j in range(QC)]

    # ---- left-stack SBUF pools (alloc order == reverse release order) ----
    pEG = tc.alloc_tile_pool(name="pEG", bufs=1)   # x_mid (residual base)
    pEF = tc.alloc_tile_pool(name="pEF", bufs=1)   # h_ln^T
    stp2 = tc.alloc_tile_pool(name="stp2", bufs=2)
    xcp = tc.alloc_tile_pool(name="xcp", bufs=2)
    wprojp = tc.alloc_tile_pool(name="wprojp", bufs=1)
    ystg = tc.alloc_tile_pool(name="ystg", bufs=4)
    dsbp = tc.alloc_tile_pool(name="dsbp", bufs=5)
    prst = tc.alloc_tile_pool(name="prst", bufs=3)
    pBC = tc.alloc_tile_pool(name="pBC", bufs=1)   # Q^T/K^T + V natural
    probs = tc.alloc_tile_pool(name="probs", bufs=6)
    pAB = tc.alloc_tile_pool(name="pAB", bufs=1)   # qkv weights
    xTp = tc.alloc_tile_pool(name="xTp", bufs=3)   # x_ln^T per-tile
    xpool = tc.alloc_tile_pool(name="xpool", bufs=6)
    stp = tc.alloc_tile_pool(name="stp", bufs=4)

    x_mid = [pEG.tile([128, C], F32, tag=f"xmid{i}", name=f"xmid{i}")
             for i in range(QC)]
    hlnT = pEF.tile([128, CB, ROWS], F16, name="hlnT")
    wproj_sb = [wprojp.tile([128, C], F16, tag=f"wp{i}", name=f"wp{i}")
                for i in range(2)]
    qkT = [pBC.tile([128, T], F16, tag=f"qkT{i}", name=f"qkT{i}")
           for i in range(4)]  # 0-1: Q^T (4 heads x 64 rows), 2-3: K^T
    vnat = [pBC.tile([128, 260], F16, tag=f"vnat{i}", name=f"vnat{i}")
            for i in range(TT)]  # per head: 64 V cols + ones col (65 each)
    wqk_sb = [pAB.tile([128, 512], F16, tag=f"wqk{i}", name=f"wqk{i}")
              for i in range(CB)]
    wv_sb = [pAB.tile([128, 256], F16, tag=f"wv{i}", name=f"wv{i}")
             for i in range(CB)]

    def rstd_of(pool, mv, tag):
        rstd = pool.tile([128, 1], F32, tag=f"rstd{tag}", name=f"rstd{tag}")
        nc.scalar.activation(out=rstd, in_=mv[:, 1:2], func=AF.Sqrt,
                             bias=epsb, scale=1.0)
        nc.vector.reciprocal(out=rstd, in_=rstd)
        return rstd

    def feed_tile(tt, xt=None):
        """Load x tile, LN1, transpose, V-natural, Q^T/K^T columns."""
        if xt is None:
            xt = xpool.tile([128, C], F16, tag="xt")  # pool bufs=6 prefetch
            nc.sync.dma_start(out=xt, in_=x16_d[tt * 128:(tt + 1) * 128, :])
        st = stp.tile([128, 2, 6], F32, tag="st")
        xr = xt.rearrange("p (g f) -> p g f", g=2)
        nc.vector.bn_stats(out=st[:, 0, :], in_=xr[:, 0, :])
        nc.vector.bn_stats(out=st[:, 1, :], in_=xr[:, 1, :])
        mv = stp.tile([128, 2], F32, tag="mv")
        nc.vector.bn_aggr(out=mv, in_=st)
        rstd = rstd_of(stp, mv, "f")
        xc = xpool.tile([128, C], F16, tag="xc", bufs=2)
        nc.vector.tensor_scalar(out=xc, in0=xt, scalar1=mv[:, 0:1],
                                scalar2=rstd, op0=OP.subtract, op1=OP.mult)
        xT = xTp.tile([128, CB, 128], F16, tag="xT")
        for cq in range(2):
            pt = psf.tile([128, 512], F16, tag="mm", name="pt")
            for i in range(4):
                cb = cq * 4 + i
                nc.tensor.matmul(
                    pt[:, 128 * i:128 * (i + 1)],
                    xc[:, cb * 128:(cb + 1) * 128], ident,
                    is_transpose=True, start=(i == 0), stop=(i == 3))
            nc.vector.tensor_copy(
                out=xT[:, cq * 4:cq * 4 + 4, :],
                in_=pt.rearrange("p (i f) -> p i f", f=128))
        # V natural
        pv = psf.tile([128, 256], F32, tag="mm", name="pv")
        for k in range(CB):
            nc.tensor.matmul(pv, xT[:, k, :], wv_sb[k],
                             start=(k == 0), stop=(k == CB - 1))
        nc.vector.tensor_copy(
            out=vnat[tt].rearrange("p (h x) -> p h x", x=65)[:, :, 64:65],
            in_=ones_c)
        nc.vector.tensor_copy(
            out=vnat[tt].rearrange("p (h x) -> p h x", x=65)[:, :, 0:64],
            in_=pv.rearrange("p (h x) -> p h x", x=64))
        # Q^T/K^T columns for this tile
        pq = psf.tile([128, 4, 128], F32, tag="mm", name="pq")
        for mt in range(4):
            for k in range(CB):
                nc.tensor.matmul(
                    pq[:, mt, :], wqk_sb[k][:, mt * 128:(mt + 1) * 128],
                    xT[:, k, :], start=(k == 0), stop=(k == CB - 1))
        for mt in range(4):
            nc.vector.tensor_scalar_add(
                out=qkT[mt][:, tt * 128:(tt + 1) * 128], in0=pq[:, mt, :],
                scalar1=bqk_sb[:, mt:mt + 1])

    # ======== attention machinery ========
    carry = []  # deferred (ysl, rbc_slice) normalizations

    def flush_one(depth=2):
        # depth-2 deferral hides the denominator DMA round trip
        if len(carry) > depth:
            ysl, rbc_sl = carry.pop(0)
            nc.vector.tensor_mul(ysl, ysl, rbc_sl)

    def flush_all():
        while carry:
            flush_one(depth=0)

    def finish_head(qc, h, py, yst_pair):
        """Denominator DRAM bounce (reciprocal taken in a [128,4] layout
        where it costs ~0.1us instead of 3.3us) + y^T staging + deferred
        normalization entry."""
        off = 64 * (h % 2)
        dtmp = dsbp.tile([65, 512], F32, tag="dtmp", bufs=3)
        nc.vector.tensor_copy(out=dtmp[0:65, :], in_=py[0:65, :])
        nc.sync.dma_start(out=dnrm[qc][h, :], in_=dtmp[64:65, :])
        drcp = dsbp.tile([128, 4], F32, tag="drcp", bufs=4)
        nc.sync.dma_start(
            out=drcp, in_=dnrm[qc][h, :].rearrange("(f p) -> p f", p=128))
        nc.vector.reciprocal(out=drcp, in_=drcp)
        nc.sync.dma_start(
            out=dnrm2[qc][h, :].rearrange("(f p) -> p f", p=128), in_=drcp)
        rbc = dsbp.tile([128, 512], F32, tag="rbc")
        nc.sync.dma_start(out=rbc[off:off + 64, :],
                          in_=_bc(dnrm2[qc][h, :], 64))
        pair = h // 2
        if h % 2 == 0:
            yst_pair[pair] = ystg.tile([128, 512], F16, tag="yst",
                                       name=f"yst{qc}_{pair}")
        ysl = yst_pair[pair][off:off + 64, :]
        nc.vector.tensor_copy(out=ysl, in_=dtmp[0:64, :])
        carry.append((ysl, rbc[off:off + 64, :]))

    class Attn3Interleaved:
        """Chunk-3 heads 0,1: key blocks processed in DESCENDING order,
        one block per step, interleaved into the feed loop."""

        def __init__(self):
            self.qc = 3
            self.py = {h: pyp.tile([128, 512], F32, tag="py",
                                   name=f"py3_{h}")
                       for h in (0, 1)}
            # full-width zero init: the accumulation group must START with
            # a write covering every element (the first real PV is partial)
            for h in (0, 1):
                nc.tensor.matmul(self.py[h][0:65, 0:512],
                                 zrow[0:1, 0:65], zrow[0:1, 0:512],
                                 start=True, stop=False)
            self.pend = {0: None, 1: None}
            self.yst_pair = [None, None]

        def step(self, kb):
            qc = self.qc
            for h in (0, 1):
                off = 64 * (h % 2)
                qh = qkT[h // 2][off:off + 64, qc * 512:(qc + 1) * 512]
                kh = qkT[2 + h // 2][off:off + 64, :]
                j = kb - 4 * qc
                lo = max(j, 0) * 128
                sr = psf.tile([128, 512], F32, tag="mm", name="sr3")
                nc.tensor.matmul(
                    sr[:, lo:512], kh[:, kb * 128:(kb + 1) * 128],
                    qh[:, lo:512], start=True, stop=True)
                pr = probs.tile([128, 512], F16, tag="pr3", bufs=4)
                nc.scalar.activation(out=pr[:, lo:512], in_=sr[:, lo:512],
                                     func=AF.Exp)
                if j >= 0:
                    nc.vector.tensor_mul(pr[:, lo:lo + 128],
                                         pr[:, lo:lo + 128], trim)
                if self.pend[h] is not None:
                    pkb, plo, ppr = self.pend[h]
                    nc.tensor.matmul(
                        self.py[h][0:65, plo:512],
                        vnat[pkb][:, h * 65:h * 65 + 65],
                        ppr[:, plo:512], start=False, stop=False)
                self.pend[h] = (kb, lo, pr)

        def finish(self):
            for h in (0, 1):
                pkb, plo, ppr = self.pend[h]
                nc.tensor.matmul(
                    self.py[h][0:65, plo:512],
                    vnat[pkb][:, h * 65:h * 65 + 65],
                    ppr[:, plo:512], start=False, stop=True)
                finish_head(3, h, self.py[h], self.yst_pair)
            return self.yst_pair

    def attention(qc, heads, yst_pair, sreg):
        """Regions of 2 key blocks (ascending); one exp per region."""
        nkb = 4 * qc + 4
        for h in heads:
            off = 64 * (h % 2)
            qh = qkT[h // 2][off:off + 64, qc * 512:(qc + 1) * 512]
            kh = qkT[2 + h // 2][off:off + 64, :]
            py = pyp.tile([128, 512], F32, tag="py", name="py")
            regions = [list(range(g, min(g + 2, nkb)))
                       for g in range(0, nkb, 2)]
            pend = None
            for ri, reg in enumerate(regions):
                # contiguous 2-bank psum region -> ONE exp instruction.
                # Both blocks are written from elo (block 1 computes a few
                # never-read columns) so the exp reads only initialized psum.
                sr = sreg.tile([128, 1024], F32, tag="sr", name="sr")
                elo = max(reg[0] - 4 * qc, 0) * 128
                for i, kb in enumerate(reg):
                    wlo = elo if i == 0 else 0
                    nc.tensor.matmul(
                        sr[:, i * 512 + wlo:(i + 1) * 512],
                        kh[:, kb * 128:(kb + 1) * 128],
                        qh[:, wlo:512], start=True, stop=True)
                pr = probs.tile([128, 1024], F16, tag="pr")
                nc.scalar.activation(out=pr[:, elo:1024],
                                     in_=sr[:, elo:1024], func=AF.Exp)
                for i, kb in enumerate(reg):
                    j = kb - 4 * qc
                    if j >= 0:
                        lo = i * 512 + 128 * j
                        nc.vector.tensor_mul(
                            pr[:, lo:lo + 128], pr[:, lo:lo + 128], trim)
                if ri == 0:
                    flush_one()
                if pend is not None:
                    for (pkb, plo, ppr, pi) in pend:
                        nc.tensor.matmul(
                            py[0:65, plo:512],
                            vnat[pkb][:, h * 65:h * 65 + 65],
                            ppr[:, pi * 512 + plo:(pi + 1) * 512],
                            start=(pkb == 0), stop=(pkb == nkb - 1))
                pend = [(kb, max(kb - 4 * qc, 0) * 128, pr, i)
                        for i, kb in enumerate(reg)]
            for (pkb, plo, ppr, pi) in pend:
                nc.tensor.matmul(
                    py[0:65, plo:512], vnat[pkb][:, h * 65:h * 65 + 65],
                    ppr[:, pi * 512 + plo:(pi + 1) * 512],
                    start=(pkb == 0), stop=(pkb == nkb - 1))
            finish_head(qc, h, py, yst_pair)

    def proj_rs(qc, yst_pair):
        """Row-parallel out-proj of chunk qc + pipelined ReduceScatter."""
        for tl in range(4):
            for cc in range(2):
                pp = paux.tile([128, 512], F32, tag="px", name="pp")
                for k in range(2):
                    nc.tensor.matmul(
                        pp, yst_pair[k][:, tl * 128:(tl + 1) * 128],
                        wproj_sb[k][:, cc * 512:(cc + 1) * 512],
                        start=(k == 0), stop=(k == 1))
                pst = prst.tile([128, 512], F16, tag="pst")
                nc.vector.tensor_copy(out=pst, in_=pp)
                nc.sync.dma_start(
                    out=attn_part[qc][tl * 128:(tl + 1) * 128,
                                      cc * 512:(cc + 1) * 512],
                    in_=pst)
        nc.gpsimd.collective_compute(
            "ReduceScatter", mybir.AluOpType.add, replica_groups=RG,
            ins=[attn_part[qc].opt()],
            outs=[rs_out[qc].opt()])

    def resid_ln2(qc, when_ms):
        # residual + LN2 + h_ln^T for the owned 128-row strip. The wait
        # hint keeps RS-dependent ops from being scheduled ahead of
        # attention work (head-of-line blocking on in-order queues).
        with tc.tile_wait_until(when_ms):
            _resid_ln2(qc)

    def _resid_ln2(qc):
        xo = xcp.tile([128, C], F32, tag="xo")
        nc.sync.dma_start(out=xo, in_=xres_d[qc * 128:(qc + 1) * 128, :])
        rst = xcp.tile([128, C], F16, tag="rst")
        nc.gpsimd.dma_start(out=rst, in_=rs_out[qc])
        nc.vector.tensor_add(x_mid[qc], rst, xo)
        st = stp2.tile([128, 2, 6], F32, tag="st2")
        xr = x_mid[qc].rearrange("p (g f) -> p g f", g=2)
        nc.vector.bn_stats(out=st[:, 0, :], in_=xr[:, 0, :])
        nc.vector.bn_stats(out=st[:, 1, :], in_=xr[:, 1, :])
        mv = stp2.tile([128, 2], F32, tag="mv2")
        nc.vector.bn_aggr(out=mv, in_=st)
        rstd = rstd_of(stp2, mv, "2")
        xc = xcp.tile([128, C], F16, tag="xc2")
        nc.vector.tensor_scalar(out=xc, in0=x_mid[qc], scalar1=mv[:, 0:1],
                                scalar2=rstd, op0=OP.subtract, op1=OP.mult)
        for cq in range(2):
            pt = paux.tile([128, 512], F16, tag="px", name="pt2")
            for i in range(4):
                cb = cq * 4 + i
                nc.tensor.matmul(
                    pt[:, 128 * i:128 * (i + 1)],
                    xc[:, cb * 128:(cb + 1) * 128], ident,
                    is_transpose=True, start=(i == 0), stop=(i == 3))
            nc.vector.tensor_copy(
                out=hlnT[:, cq * 4:cq * 4 + 4, qc * 128:(qc + 1) * 128],
                in_=pt.rearrange("p (i f) -> p i f", f=128))
        # x_mid becomes the final-residual base: fold in mlp-proj bias now
        nc.vector.tensor_add(x_mid[qc], x_mid[qc], bmp_bc)

    def fc_pass(t0, t1, h2gT, psm):
        # h2^T = gelu(wfc^T @ h_ln^T + b_fc) for row strips [t0, t1)
        n0, n1 = t0 * 128, t1 * 128
        for mg in range(4):
            for mt in range(8):
                m = mg * 8 + mt
                pf = psm.tile([128, 512], F32, tag="mm", name="pf")
                for k in range(CB):
                    nc.tensor.matmul(
                        pf[:, 0:n1 - n0],
                        wfc_sb[mg * 8 + k][:, mt * 128:(mt + 1) * 128],
                        hlnT[:, k, n0:n1], start=(k == 0),
                        stop=(k == CB - 1))
                nc.scalar.activation(
                    out=h2gT[:, m, n0:n1], in_=pf[:, 0:n1 - n0],
                    func=getattr(AF, GELU_NAME),
                    bias=bfc_sb[:, m:m + 1], scale=1.0)

    def g_pass(tlist, h2gT, psm, wmpp, outp):
        # out rows = h2g^T.T @ wmp + x_mid for the given (two) strips.
        # One [128,1024] wm DMA serves both cc halves: halves the trigger
        # count so the Sync queue can keep the weight stream ahead of PE.
        pg = {(t, cc): psm.tile([128, 512], F32, tag="mm",
                                name=f"pg{t}_{cc}")
              for t in tlist for cc in range(2)}
        for k in range(32):
            wm = wmpp.tile([128, 1024], F16, tag="wmp")
            nc.sync.dma_start(out=wm, in_=wmp_d[k * 128:(k + 1) * 128, :])
            for cc in range(2):
                for t in tlist:
                    nc.tensor.matmul(
                        pg[(t, cc)], h2gT[:, k, t * 128:(t + 1) * 128],
                        wm[:, cc * 512:(cc + 1) * 512],
                        start=(k == 0), stop=(k == 31))
        for t in tlist:
            for cc in range(2):
                ot = outp.tile([128, 512], F32, tag="ot")
                nc.vector.tensor_add(ot, pg[(t, cc)],
                                     x_mid[t][:, cc * 512:(cc + 1) * 512])
                nc.sync.dma_start(
                    out=out_d[t * 128:(t + 1) * 128,
                              cc * 512:(cc + 1) * 512],
                    in_=ot)

    # ================= program =================
    # x16 tile 15 is the very first DMA in the queue; weights go through
    # the (idle) GpSimd queue so the Sync queue serves activations. NOTE:
    # Tile dependency tracking is program-order, so every tile WRITE must
    # be emitted before any instruction that reads it — only the x DMA may
    # jump the queue, not the compute that consumes the weights.
    xts = {}
    for tt in (15, 14, 13, 12):
        xts[tt] = xpool.tile([128, C], F16, tag="xt", name=f"xt{tt}")
        nc.sync.dma_start(out=xts[tt], in_=x16_d[tt * 128:(tt + 1) * 128, :])
    for k in range(CB):
        nc.sync.dma_start(out=wv_sb[k], in_=wv_d[k * 128:(k + 1) * 128, :])
        nc.sync.dma_start(out=wqk_sb[k],
                          in_=wqk_d[k * 128:(k + 1) * 128, :])
    feed_tile(15, xt=xts[15])

    # feed descending; chunk-3 heads 0,1 interleave once their Q is ready
    A3_INTERLEAVE = True
    a3 = None
    for tt in range(TT - 2, -1, -1):
        feed_tile(tt, xt=xts.get(tt))
        if not A3_INTERLEAVE:
            continue
        if tt == 12:
            a3 = Attn3Interleaved()
            a3.step(15)
        elif a3 is not None and tt < 12:
            a3.step(tt + 3)  # kb lags the feed by 3 tiles
    if A3_INTERLEAVE:
        for kb in (2, 1, 0):
            a3.step(kb)
        yp3 = a3.finish()
    else:
        yp3 = [None, None]

    stp.release()
    xpool.release()
    xTp.release()
    pAB.release()
    psf.release()
    sreg = tc.alloc_tile_pool(name="sreg", bufs=2, space="PSUM")

    # right stack: resident wfc (loads overlap attention)
    wfcp = tc.alloc_tile_pool(name="wfcp", bufs=1, side="right")
    wfc_sb = [wfcp.tile([128, 1024], F16, tag=f"wfc{i}", name=f"wfc{i}")
              for i in range(32)]
    with tc.tile_wait_until(0.040):
        for mg in range(4):
            for k in range(CB):
                nc.scalar.dma_start(
                    out=wfc_sb[mg * 8 + k],
                    in_=wfc_d[k * 128:(k + 1) * 128,
                              mg * 1024:(mg + 1) * 1024])
    with tc.tile_wait_until(0.030):
        for k in range(2):
            nc.scalar.dma_start(out=wproj_sb[k],
                                in_=wproj_d[k * 128:(k + 1) * 128, :])
        nc.scalar.dma_start(out=bmp_bc, in_=_bc(bmp_d, 128))

    attention(3, ([2, 3] if A3_INTERLEAVE else [0, 1, 2, 3]), yp3, sreg)
    flush_all()
    proj_rs(3, yp3)
    yp2 = [None, None]
    attention(2, [0, 1, 2, 3], yp2, sreg)
    flush_all()
    proj_rs(2, yp2)
    resid_ln2(3, 0.100)
    yp1 = [None, None]
    attention(1, [0, 1, 2, 3], yp1, sreg)
    flush_all()
    proj_rs(1, yp1)
    resid_ln2(2, 0.115)
    yp0 = [None, None]
    attention(0, [0, 1, 2, 3], yp0, sreg)
    flush_all()
    proj_rs(0, yp0)
    resid_ln2(1, 0.130)
    probs.release()
    pBC.release()
    sreg.release()
    pyp.release()
    psm = tc.alloc_tile_pool(name="psm", bufs=7, space="PSUM")
    pFG = tc.alloc_tile_pool(name="pFG", bufs=1, side="right")
    wmpp = tc.alloc_tile_pool(name="wmpp", bufs=12, side="right")
    outp = tc.alloc_tile_pool(name="outp", bufs=3, side="right")
    h2gT = pFG.tile([128, 32, ROWS], F16, name="h2gT")
    with tc.tile_wait_until(0.115):
        fc_pass(2, 4, h2gT, psm)
    resid_ln2(0, 0.145)
    with tc.tile_wait_until(0.130):
        g_pass([3, 2], h2gT, psm, wmpp, outp)
    with tc.tile_wait_until(0.150):
        fc_pass(0, 2, h2gT, psm)
    with tc.tile_wait_until(0.165):
        g_pass([1, 0], h2gT, psm, wmpp, outp)

    outp.release()
    wmpp.release()
    pFG.release()
    wfcp.release()
    psm.release()
    prst.release()
    dsbp.release()
    ystg.release()
    wprojp.release()
    xcp.release()
    stp2.release()
    pEF.release()
    pEG.release()
    pyp_release = None  # (released above, before psm alloc)
    paux.release()
    dram.release()
    consts.release()


_CACHED = None


def _get_program():
    global _CACHED
    if _CACHED is None:
        _CACHED = build_program()
    return _CACHED


def _prep_inputs(inputs):
    """Fold LN params into weights and build the 8 per-core input maps."""
    x = np.asarray(inputs["x"], np.float32)
    ln1_w = np.asarray(inputs["ln1_w"], np.float32)
    ln1_b = np.asarray(inputs["ln1_b"], np.float32)
    w_attn = np.asarray(inputs["w_attn"], np.float32)
    b_attn = np.asarray(inputs["b_attn"], np.float32)
    w_proj = np.asarray(inputs["w_proj"], np.float32)
    b_proj = np.asarray(inputs["b_proj"], np.float32)
    ln2_w = np.asarray(inputs["ln2_w"], np.float32)
    ln2_b = np.asarray(inputs["ln2_b"], np.float32)
    w_fc = np.asarray(inputs["w_fc"], np.float32)
    b_fc = np.asarray(inputs["b_fc"], np.float32)
    w_mp = np.asarray(inputs["w_mlp_proj"], np.float32)
    b_mp = np.asarray(inputs["b_mlp_proj"], np.float32)

    Wa = ln1_w[:, None] * w_attn                      # [C, 3C]
    Ba = b_attn + ln1_b @ w_attn                      # [3C]
    s = 1.0 / np.sqrt(D)
    Wq = Wa[:, 0:C] * s
    Bq = Ba[0:C] * s
    Wk = Wa[:, C:2 * C]
    Bk = Ba[C:2 * C]
    Wv = Wa[:, 2 * C:3 * C]
    Bv = Ba[2 * C:3 * C]
    bproj_eff = (b_proj + Bv @ w_proj).astype(np.float32)

    Wfc = (ln2_w[:, None] * w_fc).astype(np.float32)
    Bfc = (b_fc + ln2_b @ w_fc).astype(np.float32)

    ident = np.eye(128, dtype=np.float16)
    trim = (np.arange(128)[:, None] <= np.arange(128)[None, :]).astype(
        np.float16)

    in_maps = []
    for c in range(N_CORES):
        g, p = divmod(c, TP)
        hs = slice(HPC * D * p, HPC * D * (p + 1))    # 256 cols/rows per core
        wqk = np.ascontiguousarray(
            np.concatenate([Wq[:, hs], Wk[:, hs]], axis=1), np.float16)
        bqk = np.ascontiguousarray(
            np.concatenate([Bq[hs], Bk[hs]]), np.float32)
        xres = np.concatenate(
            [x[g][512 * j + 128 * p:512 * j + 128 * p + 128]
             for j in range(QC)], axis=0) + bproj_eff[None, :]
        in_maps.append({
            "x16": np.ascontiguousarray(x[g]).astype(np.float16),
            "xres": np.ascontiguousarray(xres.astype(np.float32)),
            "wqk": wqk,
            "bqk": bqk,
            "wv": np.ascontiguousarray(Wv[:, hs]).astype(np.float16),
            "wproj": np.ascontiguousarray(w_proj[hs, :]).astype(np.float16),
            "wfc": Wfc.astype(np.float16),
            "bfc": Bfc,
            "wmp": w_mp.astype(np.float16),
            "bmp": b_mp,
            "ident": ident,
            "trim": trim,
        })
    return in_maps


def _gather(results):
    out = np.empty((B, T, C), np.float32)
    for c in range(N_CORES):
        g, p = divmod(c, TP)
        for j in range(QC):
            out[g, 512 * j + 128 * p:512 * j + 128 * p + 128, :] = \
                results[c]["out"][128 * j:128 * (j + 1)]
    return out


def kernel(**inputs) -> np.ndarray:
    nc = _get_program()
    in_maps = _prep_inputs(inputs)
    res = run_bass_kernel_spmd(nc, in_maps, list(range(N_CORES)))
    return _gather(res.results)


if __name__ == "__main__":
    print("building program...")
    _get_program()
    print("built ok")


# revision 34
# speedup vs baseline: 1.3345x; 1.3345x over previous
"""Trainium2 Bass kernel for a dense transformer block (B=2, T=2048, C=1024, H=16).

Sharding: DP2 (batch -> core groups {0-3},{4-7}) x TP4 within a group:
  - attention: Megatron head-parallel (4 heads/core), row-parallel out-proj,
    pipelined ReduceScatter(add) over the group. Query chunks processed in
    DESCENDING order (3,2,1,0); chunk 3 heads 0-1 are interleaved INTO the
    feed loop (key blocks processed in descending order as their tiles
    arrive) so the first ReduceScatter fires right after the feed finishes.
  - MLP: sequence-parallel (each core computes its 512 rows with the FULL
    fc / proj weights). No other collective.

Row ownership: core at group position p owns rows {512j+128p .. +128} for
j in 0..3 (strip j comes from query chunk j).

Device layout notes:
  - Activations feeding matmuls are kept transposed [features, tokens].
  - LN affine params folded into following weights on host; q-scale folded
    into W_q/b_q; v-bias and out-proj bias folded into xres on the host.
  - Softmax: scores^T[k,q] psum tiles; exp on ScalarE (no max subtraction:
    scores ~N(0,1)); denominator via ones-column appended to V (row 64 of
    the PV output); 1/denom via DRAM-bounce partition-broadcast (DMA
    straight out of PSUM), reciprocal taken on the broadcast [64,512],
    normalization deferred one head.
  - wfc fully resident in SBUF (right-side stack, loads overlap attention);
    wmp streamed (twice, once per g half-pass).
  - Matmul operands fp16 (full PE rate); accumulation, softmax statistics,
    residuals and LN are fp32.
"""

import os
import sys

import numpy as np

for _p in ("/opt/trn_rl_repo", "/root/.axon_site/_ro/trn_rl_repo"):
    if os.path.isdir(_p) and _p not in sys.path:
        sys.path.insert(0, _p)

import concourse.bass as bass
import concourse.tile as tile
from concourse import bacc, mybir
from concourse.bass_utils import run_bass_kernel_spmd

B, T, C, H = 2, 2048, 1024, 16
D = C // H  # 64
EPS = 1e-5
N_CORES = 8
TP = 4            # tensor-parallel group size
HPC = 4           # heads per core
ROWS = T // TP    # 512 token rows owned per core
F32 = mybir.dt.float32
F16 = mybir.dt.float16  # matmul operand dtype

TT = T // 128     # 16 token tiles
CB = C // 128     # 8 channel blocks
QC = T // 512     # 4 query chunks / row blocks
RG = [[0, 1, 2, 3], [4, 5, 6, 7]]

GELU_NAME = "Gelu_apprx_tanh"


def _bc(ap, p):
    """Broadcast a DRAM AP across p partitions (prepend stride-0 dim)."""
    return bass.AP(tensor=ap.tensor, offset=ap.offset, ap=[[0, p], *ap.ap])


def build_program():
    nc = bacc.Bacc(
        "TRN2", target_bir_lowering=False, debug=False, num_devices=N_CORES
    )

    # ---- I/O ----
    x16_d = nc.dram_tensor("x16", [T, C], F16, kind="ExternalInput").ap()
    wqk_d = nc.dram_tensor("wqk", [C, 512], F16, kind="ExternalInput").ap()
    bqk_d = nc.dram_tensor("bqk", [512], F32, kind="ExternalInput").ap()
    wv_d = nc.dram_tensor("wv", [C, 256], F16, kind="ExternalInput").ap()
    wproj_d = nc.dram_tensor("wproj", [256, C], F16, kind="ExternalInput").ap()
    wfc_d = nc.dram_tensor("wfc", [C, 4 * C], F16, kind="ExternalInput").ap()
    bfc_d = nc.dram_tensor("bfc", [4 * C], F32, kind="ExternalInput").ap()
    wmp_d = nc.dram_tensor("wmp", [4 * C, C], F16, kind="ExternalInput").ap()
    bmp_d = nc.dram_tensor("bmp", [C], F32, kind="ExternalInput").ap()
    ident_d = nc.dram_tensor("ident", [128, 128], F16, kind="ExternalInput").ap()
    trim_d = nc.dram_tensor("trim", [128, 128], F16, kind="ExternalInput").ap()
    xres_d = nc.dram_tensor("xres", [ROWS, C], F32, kind="ExternalInput").ap()
    out_d = nc.dram_tensor("out", [ROWS, C], F32, kind="ExternalOutput").ap()

    with tile.TileContext(nc) as tc:
        _body(nc, tc, locals())
    nc.compile()
    return nc


def _body(nc, tc, io):
    x16_d = io["x16_d"]; wqk_d = io["wqk_d"]; bqk_d = io["bqk_d"]
    wv_d = io["wv_d"]; wproj_d = io["wproj_d"]; wfc_d = io["wfc_d"]
    bfc_d = io["bfc_d"]; wmp_d = io["wmp_d"]; bmp_d = io["bmp_d"]
    ident_d = io["ident_d"]; trim_d = io["trim_d"]; xres_d = io["xres_d"]
    out_d = io["out_d"]

    AF = mybir.ActivationFunctionType
    OP = mybir.AluOpType

    consts = tc.alloc_tile_pool(name="consts", bufs=1)
    dram = tc.alloc_tile_pool(name="dram", bufs=1, space="DRAM")

    # ---- PSUM pools (8 banks; phased: paux1+pyp2+psf5 / +sreg4 / +psm7) --
    paux = tc.alloc_tile_pool(name="paux", bufs=1, space="PSUM")  # proj/ln2T
    pyp = tc.alloc_tile_pool(name="pyp", bufs=2, space="PSUM")    # y^T accum
    psf = tc.alloc_tile_pool(name="psf", bufs=5, space="PSUM")    # feed+attn3

    # ---------- constants ----------
    epsb = consts.tile([128, 1], F32)
    nc.vector.memset(epsb, EPS)
    ones_c = consts.tile([128, HPC, 1], F16)
    nc.vector.memset(ones_c, 1.0)
    zrow = consts.tile([1, 512], F16)
    nc.vector.memset(zrow, 0.0)
    ident = consts.tile([128, 128], F16)
    nc.sync.dma_start(out=ident, in_=ident_d)
    trim = consts.tile([128, 128], F16)
    nc.sync.dma_start(out=trim, in_=trim_d)
    bqk_sb = consts.tile([128, 4], F32)
    nc.sync.dma_start(out=bqk_sb, in_=bqk_d.rearrange("(m p) -> p m", p=128))
    bfc_sb = consts.tile([128, 32], F32)
    nc.sync.dma_start(out=bfc_sb, in_=bfc_d.rearrange("(m p) -> p m", p=128))
    bmp_bc = consts.tile([128, C], F32)

    # DRAM scratch (fp16 collective payload; per-chunk tiles avoid WAR
    # serialization between one chunk's RS read and the next chunk's
    # proj writes)
    attn_part = [dram.tile([512, C], F16, tag=f"ap{j}", name=f"ap{j}")
                 for j in range(QC)]
    rs_out = [dram.tile([128, C], F16, tag=f"rs{j}", name=f"rs{j}")
              for j in range(QC)]
    dnrm = [dram.tile([HPC, 512], F32, tag=f"dn{j}", name=f"dn{j}")
            for j in range(QC)]
    dnrm2 = [dram.tile([HPC, 512], F32, tag=f"dn2{j}", name=f"dn2{j}")
             for j in range(QC)]

    # ---- left-stack SBUF pools (alloc order == reverse release order) ----
    pEG = tc.alloc_tile_pool(name="pEG", bufs=1)   # x_mid (residual base)
    pEF = tc.alloc_tile_pool(name="pEF", bufs=1)   # h_ln^T
    stp2 = tc.alloc_tile_pool(name="stp2", bufs=2)
    xcp = tc.alloc_tile_pool(name="xcp", bufs=2)
    wprojp = tc.alloc_tile_pool(name="wprojp", bufs=1)
    ystg = tc.alloc_tile_pool(name="ystg", bufs=4)
    dsbp = tc.alloc_tile_pool(name="dsbp", bufs=5)
    prst = tc.alloc_tile_pool(name="prst", bufs=3)
    pBC = tc.alloc_tile_pool(name="pBC", bufs=1)   # Q^T/K^T + V natural
    probs = tc.alloc_tile_pool(name="probs", bufs=6)
    pAB = tc.alloc_tile_pool(name="pAB", bufs=1)   # qkv weights
    xTp = tc.alloc_tile_pool(name="xTp", bufs=3)   # x_ln^T per-tile
    xpool = tc.alloc_tile_pool(name="xpool", bufs=6)
    stp = tc.alloc_tile_pool(name="stp", bufs=4)

    x_mid = [pEG.tile([128, C], F32, tag=f"xmid{i}", name=f"xmid{i}")
             for i in range(QC)]
    hlnT = pEF.tile([128, CB, ROWS], F16, name="hlnT")
    wproj_sb = [wprojp.tile([128, C], F16, tag=f"wp{i}", name=f"wp{i}")
                for i in range(2)]
    qkT = [pBC.tile([128, T], F16, tag=f"qkT{i}", name=f"qkT{i}")
           for i in range(4)]  # 0-1: Q^T (4 heads x 64 rows), 2-3: K^T
    vnat = [pBC.tile([128, 260], F16, tag=f"vnat{i}", name=f"vnat{i}")
            for i in range(TT)]  # per head: 64 V cols + ones col (65 each)
    wqk_sb = [pAB.tile([128, 512], F16, tag=f"wqk{i}", name=f"wqk{i}")
              for i in range(CB)]
    wv_sb = [pAB.tile([128, 256], F16, tag=f"wv{i}", name=f"wv{i}")
             for i in range(CB)]

    def rstd_of(pool, mv, tag):
        rstd = pool.tile([128, 1], F32, tag=f"rstd{tag}", name=f"rstd{tag}")
        nc.scalar.activation(out=rstd, in_=mv[:, 1:2], func=AF.Sqrt,
                             bias=epsb, scale=1.0)
        nc.vector.reciprocal(out=rstd, in_=rstd)
        return rstd

    def feed_tile(tt, xt=None):
        """Load x tile, LN1, transpose, V-natural, Q^T/K^T columns."""
        if xt is None:
            xt = xpool.tile([128, C], F16, tag="xt")  # pool bufs=6 prefetch
            nc.sync.dma_start(out=xt, in_=x16_d[tt * 128:(tt + 1) * 128, :])
        st = stp.tile([128, 2, 6], F32, tag="st")
        xr = xt.rearrange("p (g f) -> p g f", g=2)
        nc.vector.bn_stats(out=st[:, 0, :], in_=xr[:, 0, :])
        nc.vector.bn_stats(out=st[:, 1, :], in_=xr[:, 1, :])
        mv = stp.tile([128, 2], F32, tag="mv")
        nc.vector.bn_aggr(out=mv, in_=st)
        rstd = rstd_of(stp, mv, "f")
        xc = xpool.tile([128, C], F16, tag="xc", bufs=2)
        nc.vector.tensor_scalar(out=xc, in0=xt, scalar1=mv[:, 0:1],
                                scalar2=rstd, op0=OP.subtract, op1=OP.mult)
        xT = xTp.tile([128, CB, 128], F16, tag="xT")
        for cq in range(2):
            pt = psf.tile([128, 512], F16, tag="mm", name="pt")
            for i in range(4):
                cb = cq * 4 + i
                nc.tensor.matmul(
                    pt[:, 128 * i:128 * (i + 1)],
                    xc[:, cb * 128:(cb + 1) * 128], ident,
                    is_transpose=True, start=(i == 0), stop=(i == 3))
            nc.vector.tensor_copy(
                out=xT[:, cq * 4:cq * 4 + 4, :],
                in_=pt.rearrange("p (i f) -> p i f", f=128))
        # V natural
        pv = psf.tile([128, 256], F32, tag="mm", name="pv")
        for k in range(CB):
            nc.tensor.matmul(pv, xT[:, k, :], wv_sb[k],
                             start=(k == 0), stop=(k == CB - 1))
        nc.vector.tensor_copy(
            out=vnat[tt].rearrange("p (h x) -> p h x", x=65)[:, :, 64:65],
            in_=ones_c)
        nc.vector.tensor_copy(
            out=vnat[tt].rearrange("p (h x) -> p h x", x=65)[:, :, 0:64],
            in_=pv.rearrange("p (h x) -> p h x", x=64))
        # Q^T/K^T columns for this tile
        pq = psf.tile([128, 4, 128], F32, tag="mm", name="pq")
        for mt in range(4):
            for k in range(CB):
                nc.tensor.matmul(
                    pq[:, mt, :], wqk_sb[k][:, mt * 128:(mt + 1) * 128],
                    xT[:, k, :], start=(k == 0), stop=(k == CB - 1))
        for mt in range(4):
            nc.vector.tensor_scalar_add(
                out=qkT[mt][:, tt * 128:(tt + 1) * 128], in0=pq[:, mt, :],
                scalar1=bqk_sb[:, mt:mt + 1])

    # ======== attention machinery ========
    carry = []  # deferred (ysl, rbc_slice) normalizations

    def flush_one(depth=0):
        if len(carry) > depth:
            ysl, rbc_sl = carry.pop(0)
            nc.vector.tensor_mul(ysl, ysl, rbc_sl)

    def flush_all():
        while carry:
            flush_one(depth=0)

    def finish_head(qc, h, py, yst_pair):
        """Denominator bounce + y^T staging + deferred-normalize entry."""
        off = 64 * (h % 2)
        dtmp = dsbp.tile([65, 512], F32, tag="dtmp", bufs=3)
        nc.vector.tensor_copy(out=dtmp[0:65, :], in_=py[0:65, :])
        nc.vector.reciprocal(out=dtmp[64:65, :], in_=dtmp[64:65, :])
        nc.sync.dma_start(out=dnrm[qc][h, :], in_=dtmp[64:65, :])
        rbc = dsbp.tile([128, 512], F32, tag="rbc")
        nc.sync.dma_start(out=rbc[off:off + 64, :],
                          in_=_bc(dnrm[qc][h, :], 64))
        pair = h // 2
        if h % 2 == 0:
            yst_pair[pair] = ystg.tile([128, 512], F16, tag="yst",
                                       name=f"yst{qc}_{pair}")
        ysl = yst_pair[pair][off:off + 64, :]
        nc.vector.tensor_copy(out=ysl, in_=dtmp[0:64, :])
        carry.append((ysl, rbc[off:off + 64, :]))

    class Attn3Interleaved:
        """Chunk-3 heads 0,1: key blocks processed in DESCENDING order,
        one block per step, interleaved into the feed loop."""

        def __init__(self):
            # key blocks processed 12, 11..0, 13, 14, 15: kb12 (diagonal
            # j=0) is full-width, so the PSUM accumulation group STARTS
            # with a write covering all 512 columns (partial-width start
            # corrupts the accumulation).
            self.qc = 3
            self.py = {h: pyp.tile([128, 512], F32, tag="py",
                                   name=f"py3_{h}")
                       for h in (0, 1)}
            self.pend = {0: None, 1: None}
            self.yst_pair = [None, None]

        def step(self, kb):
            qc = self.qc
            for h in (0, 1):
                off = 64 * (h % 2)
                qh = qkT[h // 2][off:off + 64, qc * 512:(qc + 1) * 512]
                kh = qkT[2 + h // 2][off:off + 64, :]
                j = kb - 4 * qc
                lo = max(j, 0) * 128
                sr = psf.tile([128, 512], F32, tag="mm", name="sr3")
                nc.tensor.matmul(
                    sr[:, lo:512], kh[:, kb * 128:(kb + 1) * 128],
                    qh[:, lo:512], start=True, stop=True)
                pr = probs.tile([128, 512], F16, tag="pr3", bufs=4)
                nc.scalar.activation(out=pr[:, lo:512], in_=sr[:, lo:512],
                                     func=AF.Exp)
                if j >= 0:
                    nc.vector.tensor_mul(pr[:, lo:lo + 128],
                                         pr[:, lo:lo + 128], trim)
                if self.pend[h] is not None:
                    pkb, plo, ppr = self.pend[h]
                    nc.tensor.matmul(
                        self.py[h][0:65, plo:512],
                        vnat[pkb][:, h * 65:h * 65 + 65],
                        ppr[:, plo:512], start=(pkb == 12), stop=False)
                self.pend[h] = (kb, lo, pr)

        def finish(self):
            for h in (0, 1):
                pkb, plo, ppr = self.pend[h]
                nc.tensor.matmul(
                    self.py[h][0:65, plo:512],
                    vnat[pkb][:, h * 65:h * 65 + 65],
                    ppr[:, plo:512], start=(pkb == 12), stop=True)
                finish_head(3, h, self.py[h], self.yst_pair)
            return self.yst_pair

    def attention(qc, heads, yst_pair, sreg):
        """Regions of 2 key blocks (ascending); one exp per region."""
        nkb = 4 * qc + 4
        for h in heads:
            off = 64 * (h % 2)
            qh = qkT[h // 2][off:off + 64, qc * 512:(qc + 1) * 512]
            kh = qkT[2 + h // 2][off:off + 64, :]
            py = pyp.tile([128, 512], F32, tag="py", name="py")
            regions = [list(range(g, min(g + 2, nkb)))
                       for g in range(0, nkb, 2)]
            pend = None
            for ri, reg in enumerate(regions):
                # contiguous 2-bank psum region -> ONE exp instruction.
                # Both blocks are written from elo (block 1 computes a few
                # never-read columns) so the exp reads only initialized psum.
                sr = sreg.tile([128, 1024], F32, tag="sr", name="sr")
                elo = max(reg[0] - 4 * qc, 0) * 128
                for i, kb in enumerate(reg):
                    wlo = elo if i == 0 else 0
                    nc.tensor.matmul(
                        sr[:, i * 512 + wlo:(i + 1) * 512],
                        kh[:, kb * 128:(kb + 1) * 128],
                        qh[:, wlo:512], start=True, stop=True)
                pr = probs.tile([128, 1024], F16, tag="pr")
                nc.scalar.activation(out=pr[:, elo:1024],
                                     in_=sr[:, elo:1024], func=AF.Exp)
                for i, kb in enumerate(reg):
                    j = kb - 4 * qc
                    if j >= 0:
                        lo = i * 512 + 128 * j
                        nc.vector.tensor_mul(
                            pr[:, lo:lo + 128], pr[:, lo:lo + 128], trim)
                if ri == 0:
                    flush_one()
                if pend is not None:
                    for (pkb, plo, ppr, pi) in pend:
                        nc.tensor.matmul(
                            py[0:65, plo:512],
                            vnat[pkb][:, h * 65:h * 65 + 65],
                            ppr[:, pi * 512 + plo:(pi + 1) * 512],
                            start=(pkb == 0), stop=(pkb == nkb - 1))
                pend = [(kb, max(kb - 4 * qc, 0) * 128, pr, i)
                        for i, kb in enumerate(reg)]
            for (pkb, plo, ppr, pi) in pend:
                nc.tensor.matmul(
                    py[0:65, plo:512], vnat[pkb][:, h * 65:h * 65 + 65],
                    ppr[:, pi * 512 + plo:(pi + 1) * 512],
                    start=(pkb == 0), stop=(pkb == nkb - 1))
            finish_head(qc, h, py, yst_pair)

    def proj_rs(qc, yst_pair):
        """Row-parallel out-proj of chunk qc + pipelined ReduceScatter."""
        for tl in range(4):
            for cc in range(2):
                pp = paux.tile([128, 512], F32, tag="px", name="pp")
                for k in range(2):
                    nc.tensor.matmul(
                        pp, yst_pair[k][:, tl * 128:(tl + 1) * 128],
                        wproj_sb[k][:, cc * 512:(cc + 1) * 512],
                        start=(k == 0), stop=(k == 1))
                pst = prst.tile([128, 512], F16, tag="pst")
                nc.vector.tensor_copy(out=pst, in_=pp)
                nc.sync.dma_start(
                    out=attn_part[qc][tl * 128:(tl + 1) * 128,
                                      cc * 512:(cc + 1) * 512],
                    in_=pst)
        nc.gpsimd.collective_compute(
            "ReduceScatter", mybir.AluOpType.add, replica_groups=RG,
            ins=[attn_part[qc].opt()],
            outs=[rs_out[qc].opt()])

    def resid_ln2(qc, when_ms):
        # residual + LN2 + h_ln^T for the owned 128-row strip. The wait
        # hint keeps RS-dependent ops from being scheduled ahead of
        # attention work (head-of-line blocking on in-order queues).
        with tc.tile_wait_until(when_ms):
            _resid_ln2(qc)

    def _resid_ln2(qc):
        xo = xcp.tile([128, C], F32, tag="xo")
        nc.sync.dma_start(out=xo, in_=xres_d[qc * 128:(qc + 1) * 128, :])
        rst = xcp.tile([128, C], F16, tag="rst")
        nc.gpsimd.dma_start(out=rst, in_=rs_out[qc])
        nc.vector.tensor_add(x_mid[qc], rst, xo)
        st = stp2.tile([128, 2, 6], F32, tag="st2")
        xr = x_mid[qc].rearrange("p (g f) -> p g f", g=2)
        nc.vector.bn_stats(out=st[:, 0, :], in_=xr[:, 0, :])
        nc.vector.bn_stats(out=st[:, 1, :], in_=xr[:, 1, :])
        mv = stp2.tile([128, 2], F32, tag="mv2")
        nc.vector.bn_aggr(out=mv, in_=st)
        rstd = rstd_of(stp2, mv, "2")
        xc = xcp.tile([128, C], F16, tag="xc2")
        nc.vector.tensor_scalar(out=xc, in0=x_mid[qc], scalar1=mv[:, 0:1],
                                scalar2=rstd, op0=OP.subtract, op1=OP.mult)
        for cq in range(2):
            pt = paux.tile([128, 512], F16, tag="px", name="pt2")
            for i in range(4):
                cb = cq * 4 + i
                nc.tensor.matmul(
                    pt[:, 128 * i:128 * (i + 1)],
                    xc[:, cb * 128:(cb + 1) * 128], ident,
                    is_transpose=True, start=(i == 0), stop=(i == 3))
            nc.vector.tensor_copy(
                out=hlnT[:, cq * 4:cq * 4 + 4, qc * 128:(qc + 1) * 128],
                in_=pt.rearrange("p (i f) -> p i f", f=128))
        # x_mid becomes the final-residual base: fold in mlp-proj bias now
        nc.vector.tensor_add(x_mid[qc], x_mid[qc], bmp_bc)

    def fc_pass(t0, t1, h2gT, psm):
        # h2^T = gelu(wfc^T @ h_ln^T + b_fc) for row strips [t0, t1)
        n0, n1 = t0 * 128, t1 * 128
        for mg in range(4):
            for mt in range(8):
                m = mg * 8 + mt
                pf = psm.tile([128, 512], F32, tag="mm", name="pf")
                for k in range(CB):
                    nc.tensor.matmul(
                        pf[:, 0:n1 - n0],
                        wfc_sb[mg * 8 + k][:, mt * 128:(mt + 1) * 128],
                        hlnT[:, k, n0:n1], start=(k == 0),
                        stop=(k == CB - 1))
                nc.scalar.activation(
                    out=h2gT[:, m, n0:n1], in_=pf[:, 0:n1 - n0],
                    func=getattr(AF, GELU_NAME),
                    bias=bfc_sb[:, m:m + 1], scale=1.0)

    def g_pass(tlist, h2gT, psm, wmpp, outp):
        # out rows = h2g^T.T @ wmp + x_mid for the given (two) strips.
        # One [128,1024] wm DMA serves both cc halves: halves the trigger
        # count so the Sync queue can keep the weight stream ahead of PE.
        pg = {(t, cc): psm.tile([128, 512], F32, tag="mm",
                                name=f"pg{t}_{cc}")
              for t in tlist for cc in range(2)}
        for k in range(32):
            wm = wmpp.tile([128, 1024], F16, tag="wmp")
            nc.sync.dma_start(out=wm, in_=wmp_d[k * 128:(k + 1) * 128, :])
            for cc in range(2):
                for t in tlist:
                    nc.tensor.matmul(
                        pg[(t, cc)], h2gT[:, k, t * 128:(t + 1) * 128],
                        wm[:, cc * 512:(cc + 1) * 512],
                        start=(k == 0), stop=(k == 31))
        for t in tlist:
            for cc in range(2):
                ot = outp.tile([128, 512], F32, tag="ot")
                nc.vector.tensor_add(ot, pg[(t, cc)],
                                     x_mid[t][:, cc * 512:(cc + 1) * 512])
                nc.sync.dma_start(
                    out=out_d[t * 128:(t + 1) * 128,
                              cc * 512:(cc + 1) * 512],
                    in_=ot)

    # ================= program =================
    # x16 tile 15 is the very first DMA in the queue; weights go through
    # the (idle) GpSimd queue so the Sync queue serves activations. NOTE:
    # Tile dependency tracking is program-order, so every tile WRITE must
    # be emitted before any instruction that reads it — only the x DMA may
    # jump the queue, not the compute that consumes the weights.
    xts = {}
    for tt in (15, 14, 13, 12):
        xts[tt] = xpool.tile([128, C], F16, tag="xt", name=f"xt{tt}")
        nc.sync.dma_start(out=xts[tt], in_=x16_d[tt * 128:(tt + 1) * 128, :])
    for k in range(CB):
        nc.sync.dma_start(out=wv_sb[k], in_=wv_d[k * 128:(k + 1) * 128, :])
        nc.sync.dma_start(out=wqk_sb[k],
                          in_=wqk_d[k * 128:(k + 1) * 128, :])
    feed_tile(15, xt=xts[15])

    # feed descending; chunk-3 heads 0,1 interleave once their Q is ready
    A3_INTERLEAVE = True
    a3 = None
    for tt in range(TT - 2, -1, -1):
        feed_tile(tt, xt=xts.get(tt))
        if not A3_INTERLEAVE:
            continue
        if tt == 12:
            a3 = Attn3Interleaved()
            a3.step(12)
        elif a3 is not None and tt < 12:
            a3.step(tt)  # kb = the tile just fed
    if A3_INTERLEAVE:
        for kb in (13, 14, 15):
            a3.step(kb)
        yp3 = a3.finish()
    else:
        yp3 = [None, None]

    stp.release()
    xpool.release()
    xTp.release()
    pAB.release()
    psf.release()
    sreg = tc.alloc_tile_pool(name="sreg", bufs=2, space="PSUM")

    # right stack: resident wfc (loads overlap attention)
    wfcp = tc.alloc_tile_pool(name="wfcp", bufs=1, side="right")
    wfc_sb = [wfcp.tile([128, 1024], F16, tag=f"wfc{i}", name=f"wfc{i}")
              for i in range(32)]
    with tc.tile_wait_until(0.040):
        for mg in range(4):
            for k in range(CB):
                nc.scalar.dma_start(
                    out=wfc_sb[mg * 8 + k],
                    in_=wfc_d[k * 128:(k + 1) * 128,
                              mg * 1024:(mg + 1) * 1024])
    with tc.tile_wait_until(0.030):
        for k in range(2):
            nc.scalar.dma_start(out=wproj_sb[k],
                                in_=wproj_d[k * 128:(k + 1) * 128, :])
        nc.scalar.dma_start(out=bmp_bc, in_=_bc(bmp_d, 128))

    attention(3, ([2, 3] if A3_INTERLEAVE else [0, 1, 2, 3]), yp3, sreg)
    flush_all()
    proj_rs(3, yp3)
    yp2 = [None, None]
    attention(2, [0, 1, 2, 3], yp2, sreg)
    flush_all()
    proj_rs(2, yp2)
    resid_ln2(3, 0.100)
    yp1 = [None, None]
    attention(1, [0, 1, 2, 3], yp1, sreg)
    flush_all()
    proj_rs(1, yp1)
    resid_ln2(2, 0.115)
    yp0 = [None, None]
    attention(0, [0, 1, 2, 3], yp0, sreg)
    flush_all()
    proj_rs(0, yp0)
    resid_ln2(1, 0.130)
    probs.release()
    pBC.release()
    sreg.release()
    pyp.release()
    psm = tc.alloc_tile_pool(name="psm", bufs=7, space="PSUM")
    pFG = tc.alloc_tile_pool(name="pFG", bufs=1, side="right")
    wmpp = tc.alloc_tile_pool(name="wmpp", bufs=12, side="right")
    outp = tc.alloc_tile_pool(name="outp", bufs=3, side="right")
    h2gT = pFG.tile([128, 32, ROWS], F16, name="h2gT")
    with tc.tile_wait_until(0.115):
        fc_pass(2, 4, h2gT, psm)
    resid_ln2(0, 0.145)
    with tc.tile_wait_until(0.130):
        g_pass([3, 2], h2gT, psm, wmpp, outp)
    with tc.tile_wait_until(0.150):
        fc_pass(0, 2, h2gT, psm)
    with tc.tile_wait_until(0.165):
        g_pass([1, 0], h2gT, psm, wmpp, outp)

    outp.release()
    wmpp.release()
    pFG.release()
    wfcp.release()
    psm.release()
    prst.release()
    dsbp.release()
    ystg.release()
    wprojp.release()
    xcp.release()
    stp2.release()
    pEF.release()
    pEG.release()
    pyp_release = None  # (released above, before psm alloc)
    paux.release()
    dram.release()
    consts.release()


_CACHED = None


def _get_program():
    global _CACHED
    if _CACHED is None:
        _CACHED = build_program()
    return _CACHED


def _prep_inputs(inputs):
    """Fold LN params into weights and build the 8 per-core input maps."""
    x = np.asarray(inputs["x"], np.float32)
    ln1_w = np.asarray(inputs["ln1_w"], np.float32)
    ln1_b = np.asarray(inputs["ln1_b"], np.float32)
    w_attn = np.asarray(inputs["w_attn"], np.float32)
    b_attn = np.asarray(inputs["b_attn"], np.float32)
    w_proj = np.asarray(inputs["w_proj"], np.float32)
    b_proj = np.asarray(inputs["b_proj"], np.float32)
    ln2_w = np.asarray(inputs["ln2_w"], np.float32)
    ln2_b = np.asarray(inputs["ln2_b"], np.float32)
    w_fc = np.asarray(inputs["w_fc"], np.float32)
    b_fc = np.asarray(inputs["b_fc"], np.float32)
    w_mp = np.asarray(inputs["w_mlp_proj"], np.float32)
    b_mp = np.asarray(inputs["b_mlp_proj"], np.float32)

    Wa = ln1_w[:, None] * w_attn                      # [C, 3C]
    Ba = b_attn + ln1_b @ w_attn                      # [3C]
    s = 1.0 / np.sqrt(D)
    Wq = Wa[:, 0:C] * s
    Bq = Ba[0:C] * s
    Wk = Wa[:, C:2 * C]
    Bk = Ba[C:2 * C]
    Wv = Wa[:, 2 * C:3 * C]
    Bv = Ba[2 * C:3 * C]
    bproj_eff = (b_proj + Bv @ w_proj).astype(np.float32)

    Wfc = (ln2_w[:, None] * w_fc).astype(np.float32)
    Bfc = (b_fc + ln2_b @ w_fc).astype(np.float32)

    ident = np.eye(128, dtype=np.float16)
    trim = (np.arange(128)[:, None] <= np.arange(128)[None, :]).astype(
        np.float16)

    in_maps = []
    for c in range(N_CORES):
        g, p = divmod(c, TP)
        hs = slice(HPC * D * p, HPC * D * (p + 1))    # 256 cols/rows per core
        wqk = np.ascontiguousarray(
            np.concatenate([Wq[:, hs], Wk[:, hs]], axis=1), np.float16)
        bqk = np.ascontiguousarray(
            np.concatenate([Bq[hs], Bk[hs]]), np.float32)
        xres = np.concatenate(
            [x[g][512 * j + 128 * p:512 * j + 128 * p + 128]
             for j in range(QC)], axis=0) + bproj_eff[None, :]
        in_maps.append({
            "x16": np.ascontiguousarray(x[g]).astype(np.float16),
            "xres": np.ascontiguousarray(xres.astype(np.float32)),
            "wqk": wqk,
            "bqk": bqk,
            "wv": np.ascontiguousarray(Wv[:, hs]).astype(np.float16),
            "wproj": np.ascontiguousarray(w_proj[hs, :]).astype(np.float16),
            "wfc": Wfc.astype(np.float16),
            "bfc": Bfc,
            "wmp": w_mp.astype(np.float16),
            "bmp": b_mp,
            "ident": ident,
            "trim": trim,
        })
    return in_maps


def _gather(results):
    out = np.empty((B, T, C), np.float32)
    for c in range(N_CORES):
        g, p = divmod(c, TP)
        for j in range(QC):
            out[g, 512 * j + 128 * p:512 * j + 128 * p + 128, :] = \
                results[c]["out"][128 * j:128 * (j + 1)]
    return out


def kernel(**inputs) -> np.ndarray:
    nc = _get_program()
    in_maps = _prep_inputs(inputs)
    res = run_bass_kernel_spmd(nc, in_maps, list(range(N_CORES)))
    return _gather(res.results)


if __name__ == "__main__":
    print("building program...")
    _get_program()
    print("built ok")


# revision 35
# speedup vs baseline: 1.4228x; 1.0662x over previous
"""Trainium2 Bass kernel for a dense transformer block (B=2, T=2048, C=1024, H=16).

Sharding: DP2 (batch -> core groups {0-3},{4-7}) x TP4 within a group:
  - attention: Megatron head-parallel (4 heads/core), row-parallel out-proj,
    pipelined ReduceScatter(add) over the group. Query chunks processed in
    DESCENDING order (3,2,1,0); chunk 3 heads 0-1 are interleaved INTO the
    feed loop (key blocks processed in descending order as their tiles
    arrive) so the first ReduceScatter fires right after the feed finishes.
  - MLP: sequence-parallel (each core computes its 512 rows with the FULL
    fc / proj weights). No other collective.

Row ownership: core at group position p owns rows {512j+128p .. +128} for
j in 0..3 (strip j comes from query chunk j).

Device layout notes:
  - Activations feeding matmuls are kept transposed [features, tokens].
  - LN affine params folded into following weights on host; q-scale folded
    into W_q/b_q; v-bias and out-proj bias folded into xres on the host.
  - Softmax: scores^T[k,q] psum tiles; exp on ScalarE (no max subtraction:
    scores ~N(0,1)); denominator via ones-column appended to V (row 64 of
    the PV output); 1/denom via DRAM-bounce partition-broadcast (DMA
    straight out of PSUM), reciprocal taken on the broadcast [64,512],
    normalization deferred one head.
  - wfc fully resident in SBUF (right-side stack, loads overlap attention);
    wmp streamed (twice, once per g half-pass).
  - Matmul operands fp16 (full PE rate); accumulation, softmax statistics,
    residuals and LN are fp32.
"""

import os
import sys

import numpy as np

for _p in ("/opt/trn_rl_repo", "/root/.axon_site/_ro/trn_rl_repo"):
    if os.path.isdir(_p) and _p not in sys.path:
        sys.path.insert(0, _p)

import concourse.bass as bass
import concourse.tile as tile
from concourse import bacc, mybir
from concourse.bass_utils import run_bass_kernel_spmd

B, T, C, H = 2, 2048, 1024, 16
D = C // H  # 64
EPS = 1e-5
N_CORES = 8
TP = 4            # tensor-parallel group size
HPC = 4           # heads per core
ROWS = T // TP    # 512 token rows owned per core
F32 = mybir.dt.float32
F16 = mybir.dt.float16  # matmul operand dtype

TT = T // 128     # 16 token tiles
CB = C // 128     # 8 channel blocks
QC = T // 512     # 4 query chunks / row blocks
RG = [[0, 1, 2, 3], [4, 5, 6, 7]]

GELU_NAME = "Gelu_apprx_tanh"


def _bc(ap, p):
    """Broadcast a DRAM AP across p partitions (prepend stride-0 dim)."""
    return bass.AP(tensor=ap.tensor, offset=ap.offset, ap=[[0, p], *ap.ap])


def build_program():
    nc = bacc.Bacc(
        "TRN2", target_bir_lowering=False, debug=False, num_devices=N_CORES
    )

    # ---- I/O ----
    x16_d = nc.dram_tensor("x16", [T, C], F16, kind="ExternalInput").ap()
    wqk_d = nc.dram_tensor("wqk", [C, 512], F16, kind="ExternalInput").ap()
    bqk_d = nc.dram_tensor("bqk", [512], F32, kind="ExternalInput").ap()
    wv_d = nc.dram_tensor("wv", [C, 256], F16, kind="ExternalInput").ap()
    wproj_d = nc.dram_tensor("wproj", [256, C], F16, kind="ExternalInput").ap()
    wfc_d = nc.dram_tensor("wfc", [C, 4 * C], F16, kind="ExternalInput").ap()
    bfc_d = nc.dram_tensor("bfc", [4 * C], F32, kind="ExternalInput").ap()
    wmp_d = nc.dram_tensor("wmp", [4 * C, C], F16, kind="ExternalInput").ap()
    bmp_d = nc.dram_tensor("bmp", [C], F32, kind="ExternalInput").ap()
    ident_d = nc.dram_tensor("ident", [128, 128], F16, kind="ExternalInput").ap()
    trim_d = nc.dram_tensor("trim", [128, 128], F16, kind="ExternalInput").ap()
    xres_d = nc.dram_tensor("xres", [ROWS, C], F32, kind="ExternalInput").ap()
    out_d = nc.dram_tensor("out", [ROWS, C], F32, kind="ExternalOutput").ap()

    with tile.TileContext(nc) as tc:
        _body(nc, tc, locals())
    nc.compile()
    return nc


def _body(nc, tc, io):
    x16_d = io["x16_d"]; wqk_d = io["wqk_d"]; bqk_d = io["bqk_d"]
    wv_d = io["wv_d"]; wproj_d = io["wproj_d"]; wfc_d = io["wfc_d"]
    bfc_d = io["bfc_d"]; wmp_d = io["wmp_d"]; bmp_d = io["bmp_d"]
    ident_d = io["ident_d"]; trim_d = io["trim_d"]; xres_d = io["xres_d"]
    out_d = io["out_d"]

    AF = mybir.ActivationFunctionType
    OP = mybir.AluOpType

    consts = tc.alloc_tile_pool(name="consts", bufs=1)
    dram = tc.alloc_tile_pool(name="dram", bufs=1, space="DRAM")

    # ---- PSUM pools (8 banks; phased: paux1+pyp2+psf5 / +sreg4 / +psm7) --
    paux = tc.alloc_tile_pool(name="paux", bufs=1, space="PSUM")  # proj/ln2T
    pyp = tc.alloc_tile_pool(name="pyp", bufs=2, space="PSUM")    # y^T accum
    psf = tc.alloc_tile_pool(name="psf", bufs=5, space="PSUM")    # feed+attn3

    # ---------- constants ----------
    epsb = consts.tile([128, 1], F32)
    nc.vector.memset(epsb, EPS)
    ones_c = consts.tile([128, HPC, 1], F16)
    nc.vector.memset(ones_c, 1.0)
    zrow = consts.tile([1, 512], F16)
    nc.vector.memset(zrow, 0.0)
    ident = consts.tile([128, 128], F16)
    nc.sync.dma_start(out=ident, in_=ident_d)
    trim = consts.tile([128, 128], F16)
    nc.sync.dma_start(out=trim, in_=trim_d)
    bqk_sb = consts.tile([128, 4], F32)
    nc.sync.dma_start(out=bqk_sb, in_=bqk_d.rearrange("(m p) -> p m", p=128))
    bfc_sb = consts.tile([128, 32], F32)
    nc.sync.dma_start(out=bfc_sb, in_=bfc_d.rearrange("(m p) -> p m", p=128))
    bmp_bc = consts.tile([128, C], F32)

    # DRAM scratch (fp16 collective payload; per-chunk tiles avoid WAR
    # serialization between one chunk's RS read and the next chunk's
    # proj writes)
    attn_part = [dram.tile([512, C], F16, tag=f"ap{j}", name=f"ap{j}")
                 for j in range(QC)]
    rs_out = [dram.tile([128, C], F16, tag=f"rs{j}", name=f"rs{j}")
              for j in range(QC)]
    dnrm = [dram.tile([HPC, 512], F32, tag=f"dn{j}", name=f"dn{j}")
            for j in range(QC)]
    dnrm2 = [dram.tile([HPC, 512], F32, tag=f"dn2{j}", name=f"dn2{j}")
             for j in range(QC)]

    # ---- left-stack SBUF pools (alloc order == reverse release order) ----
    pEG = tc.alloc_tile_pool(name="pEG", bufs=1)   # x_mid (residual base)
    pEF = tc.alloc_tile_pool(name="pEF", bufs=1)   # h_ln^T
    stp2 = tc.alloc_tile_pool(name="stp2", bufs=2)
    xcp = tc.alloc_tile_pool(name="xcp", bufs=2)
    wprojp = tc.alloc_tile_pool(name="wprojp", bufs=1)
    ystg = tc.alloc_tile_pool(name="ystg", bufs=4)
    dsbp = tc.alloc_tile_pool(name="dsbp", bufs=5)
    prst = tc.alloc_tile_pool(name="prst", bufs=3)
    pBC = tc.alloc_tile_pool(name="pBC", bufs=1)   # Q^T/K^T + V natural
    probs = tc.alloc_tile_pool(name="probs", bufs=6)
    pAB = tc.alloc_tile_pool(name="pAB", bufs=1)   # qkv weights
    xTp = tc.alloc_tile_pool(name="xTp", bufs=3)   # x_ln^T per-tile
    xpool = tc.alloc_tile_pool(name="xpool", bufs=6)
    stp = tc.alloc_tile_pool(name="stp", bufs=4)

    x_mid = [pEG.tile([128, C], F32, tag=f"xmid{i}", name=f"xmid{i}")
             for i in range(QC)]
    hlnT = pEF.tile([128, CB, ROWS], F16, name="hlnT")
    wproj_sb = [wprojp.tile([128, C], F16, tag=f"wp{i}", name=f"wp{i}")
                for i in range(2)]
    qkT = [pBC.tile([128, T], F16, tag=f"qkT{i}", name=f"qkT{i}")
           for i in range(4)]  # 0-1: Q^T (4 heads x 64 rows), 2-3: K^T
    vnat = [pBC.tile([128, 260], F16, tag=f"vnat{i}", name=f"vnat{i}")
            for i in range(TT)]  # per head: 64 V cols + ones col (65 each)
    wqk_sb = [pAB.tile([128, 512], F16, tag=f"wqk{i}", name=f"wqk{i}")
              for i in range(CB)]
    wv_sb = [pAB.tile([128, 256], F16, tag=f"wv{i}", name=f"wv{i}")
             for i in range(CB)]

    def rstd_of(pool, mv, tag):
        rstd = pool.tile([128, 1], F32, tag=f"rstd{tag}", name=f"rstd{tag}")
        nc.scalar.activation(out=rstd, in_=mv[:, 1:2], func=AF.Sqrt,
                             bias=epsb, scale=1.0)
        nc.vector.reciprocal(out=rstd, in_=rstd)
        return rstd

    def feed_tile(tt, xt=None):
        """Load x tile, LN1, transpose, V-natural, Q^T/K^T columns."""
        if xt is None:
            xt = xpool.tile([128, C], F16, tag="xt")  # pool bufs=6 prefetch
            nc.sync.dma_start(out=xt, in_=x16_d[tt * 128:(tt + 1) * 128, :])
        st = stp.tile([128, 2, 6], F32, tag="st")
        xr = xt.rearrange("p (g f) -> p g f", g=2)
        nc.vector.bn_stats(out=st[:, 0, :], in_=xr[:, 0, :])
        nc.vector.bn_stats(out=st[:, 1, :], in_=xr[:, 1, :])
        mv = stp.tile([128, 2], F32, tag="mv")
        nc.vector.bn_aggr(out=mv, in_=st)
        rstd = rstd_of(stp, mv, "f")
        xc = xpool.tile([128, C], F16, tag="xc", bufs=2)
        nc.vector.tensor_scalar(out=xc, in0=xt, scalar1=mv[:, 0:1],
                                scalar2=rstd, op0=OP.subtract, op1=OP.mult)
        xT = xTp.tile([128, CB, 128], F16, tag="xT")
        for cq in range(2):
            pt = psf.tile([128, 512], F16, tag="mm", name="pt")
            for i in range(4):
                cb = cq * 4 + i
                nc.tensor.matmul(
                    pt[:, 128 * i:128 * (i + 1)],
                    xc[:, cb * 128:(cb + 1) * 128], ident,
                    is_transpose=True, start=(i == 0), stop=(i == 3))
            nc.vector.tensor_copy(
                out=xT[:, cq * 4:cq * 4 + 4, :],
                in_=pt.rearrange("p (i f) -> p i f", f=128))
        # V natural
        pv = psf.tile([128, 256], F32, tag="mm", name="pv")
        for k in range(CB):
            nc.tensor.matmul(pv, xT[:, k, :], wv_sb[k],
                             start=(k == 0), stop=(k == CB - 1))
        nc.vector.tensor_copy(
            out=vnat[tt].rearrange("p (h x) -> p h x", x=65)[:, :, 64:65],
            in_=ones_c)
        nc.vector.tensor_copy(
            out=vnat[tt].rearrange("p (h x) -> p h x", x=65)[:, :, 0:64],
            in_=pv.rearrange("p (h x) -> p h x", x=64))
        # Q^T/K^T columns for this tile
        pq = psf.tile([128, 4, 128], F32, tag="mm", name="pq")
        for mt in range(4):
            for k in range(CB):
                nc.tensor.matmul(
                    pq[:, mt, :], wqk_sb[k][:, mt * 128:(mt + 1) * 128],
                    xT[:, k, :], start=(k == 0), stop=(k == CB - 1))
        for mt in range(4):
            nc.vector.tensor_scalar_add(
                out=qkT[mt][:, tt * 128:(tt + 1) * 128], in0=pq[:, mt, :],
                scalar1=bqk_sb[:, mt:mt + 1])

    # ======== attention machinery ========
    carry = []  # deferred (ysl, rbc_slice) normalizations

    def flush_one(depth=0):
        if len(carry) > depth:
            ysl, rbc_sl = carry.pop(0)
            nc.vector.tensor_mul(ysl, ysl, rbc_sl)

    def flush_all():
        while carry:
            flush_one(depth=0)

    def finish_head(qc, h, py, yst_pair):
        """Denominator bounce + y^T staging + deferred-normalize entry."""
        off = 64 * (h % 2)
        dtmp = dsbp.tile([65, 512], F32, tag="dtmp", bufs=3)
        nc.vector.tensor_copy(out=dtmp[0:65, :], in_=py[0:65, :])
        nc.vector.reciprocal(out=dtmp[64:65, :], in_=dtmp[64:65, :])
        nc.sync.dma_start(out=dnrm[qc][h, :], in_=dtmp[64:65, :])
        rbc = dsbp.tile([128, 512], F32, tag="rbc")
        nc.sync.dma_start(out=rbc[off:off + 64, :],
                          in_=_bc(dnrm[qc][h, :], 64))
        pair = h // 2
        if h % 2 == 0:
            yst_pair[pair] = ystg.tile([128, 512], F16, tag="yst",
                                       name=f"yst{qc}_{pair}")
        ysl = yst_pair[pair][off:off + 64, :]
        nc.vector.tensor_copy(out=ysl, in_=dtmp[0:64, :])
        carry.append((ysl, rbc[off:off + 64, :]))

    class Attn3Interleaved:
        """Chunk-3 heads 0,1: key blocks processed in DESCENDING order,
        one block per step, interleaved into the feed loop."""

        def __init__(self):
            # key blocks processed 12, 11..0, 13, 14, 15: kb12 (diagonal
            # j=0) is full-width, so the PSUM accumulation group STARTS
            # with a write covering all 512 columns (partial-width start
            # corrupts the accumulation).
            self.qc = 3
            self.py = {h: pyp.tile([128, 512], F32, tag="py",
                                   name=f"py3_{h}")
                       for h in (0, 1)}
            self.pend = {0: None, 1: None}
            self.yst_pair = [None, None]

        def step(self, kb):
            qc = self.qc
            for h in (0, 1):
                off = 64 * (h % 2)
                qh = qkT[h // 2][off:off + 64, qc * 512:(qc + 1) * 512]
                kh = qkT[2 + h // 2][off:off + 64, :]
                j = kb - 4 * qc
                lo = max(j, 0) * 128
                sr = psf.tile([128, 512], F32, tag="mm", name="sr3")
                nc.tensor.matmul(
                    sr[:, lo:512], kh[:, kb * 128:(kb + 1) * 128],
                    qh[:, lo:512], start=True, stop=True)
                pr = probs.tile([128, 512], F16, tag="pr3", bufs=4)
                nc.scalar.activation(out=pr[:, lo:512], in_=sr[:, lo:512],
                                     func=AF.Exp)
                if j >= 0:
                    nc.vector.tensor_mul(pr[:, lo:lo + 128],
                                         pr[:, lo:lo + 128], trim)
                if self.pend[h] is not None:
                    pkb, plo, ppr = self.pend[h]
                    nc.tensor.matmul(
                        self.py[h][0:65, plo:512],
                        vnat[pkb][:, h * 65:h * 65 + 65],
                        ppr[:, plo:512], start=(pkb == 12), stop=False)
                self.pend[h] = (kb, lo, pr)

        def finish(self):
            for h in (0, 1):
                pkb, plo, ppr = self.pend[h]
                nc.tensor.matmul(
                    self.py[h][0:65, plo:512],
                    vnat[pkb][:, h * 65:h * 65 + 65],
                    ppr[:, plo:512], start=(pkb == 12), stop=True)
                finish_head(3, h, self.py[h], self.yst_pair)
            return self.yst_pair

    def attention(qc, heads, yst_pair, sreg):
        """Regions of 2 key blocks (ascending); one exp per region."""
        nkb = 4 * qc + 4
        for h in heads:
            off = 64 * (h % 2)
            qh = qkT[h // 2][off:off + 64, qc * 512:(qc + 1) * 512]
            kh = qkT[2 + h // 2][off:off + 64, :]
            py = pyp.tile([128, 512], F32, tag="py", name="py")
            regions = [list(range(g, min(g + 2, nkb)))
                       for g in range(0, nkb, 2)]
            pend = None
            for ri, reg in enumerate(regions):
                # contiguous 2-bank psum region -> ONE exp instruction.
                # Both blocks are written from elo (block 1 computes a few
                # never-read columns) so the exp reads only initialized psum.
                sr = sreg.tile([128, 1024], F32, tag="sr", name="sr")
                elo = max(reg[0] - 4 * qc, 0) * 128
                for i, kb in enumerate(reg):
                    wlo = elo if i == 0 else 0
                    nc.tensor.matmul(
                        sr[:, i * 512 + wlo:(i + 1) * 512],
                        kh[:, kb * 128:(kb + 1) * 128],
                        qh[:, wlo:512], start=True, stop=True)
                pr = probs.tile([128, 1024], F16, tag="pr")
                nc.scalar.activation(out=pr[:, elo:1024],
                                     in_=sr[:, elo:1024], func=AF.Exp)
                for i, kb in enumerate(reg):
                    j = kb - 4 * qc
                    if j >= 0:
                        lo = i * 512 + 128 * j
                        nc.vector.tensor_mul(
                            pr[:, lo:lo + 128], pr[:, lo:lo + 128], trim)
                if ri == 0:
                    flush_one()
                if pend is not None:
                    for (pkb, plo, ppr, pi) in pend:
                        nc.tensor.matmul(
                            py[0:65, plo:512],
                            vnat[pkb][:, h * 65:h * 65 + 65],
                            ppr[:, pi * 512 + plo:(pi + 1) * 512],
                            start=(pkb == 0), stop=(pkb == nkb - 1))
                pend = [(kb, max(kb - 4 * qc, 0) * 128, pr, i)
                        for i, kb in enumerate(reg)]
            for (pkb, plo, ppr, pi) in pend:
                nc.tensor.matmul(
                    py[0:65, plo:512], vnat[pkb][:, h * 65:h * 65 + 65],
                    ppr[:, pi * 512 + plo:(pi + 1) * 512],
                    start=(pkb == 0), stop=(pkb == nkb - 1))
            finish_head(qc, h, py, yst_pair)

    def proj_rs(qc, yst_pair):
        """Row-parallel out-proj of chunk qc + pipelined ReduceScatter."""
        for tl in range(4):
            for cc in range(2):
                pp = paux.tile([128, 512], F32, tag="px", name="pp")
                for k in range(2):
                    nc.tensor.matmul(
                        pp, yst_pair[k][:, tl * 128:(tl + 1) * 128],
                        wproj_sb[k][:, cc * 512:(cc + 1) * 512],
                        start=(k == 0), stop=(k == 1))
                pst = prst.tile([128, 512], F16, tag="pst")
                nc.vector.tensor_copy(out=pst, in_=pp)
                nc.sync.dma_start(
                    out=attn_part[qc][tl * 128:(tl + 1) * 128,
                                      cc * 512:(cc + 1) * 512],
                    in_=pst)
        nc.gpsimd.collective_compute(
            "ReduceScatter", mybir.AluOpType.add, replica_groups=RG,
            ins=[attn_part[qc].opt()],
            outs=[rs_out[qc].opt()])

    def resid_ln2(qc, when_ms):
        # residual + LN2 + h_ln^T for the owned 128-row strip. The wait
        # hint keeps RS-dependent ops from being scheduled ahead of
        # attention work (head-of-line blocking on in-order queues).
        with tc.tile_wait_until(when_ms):
            _resid_ln2(qc)

    def _resid_ln2(qc):
        xo = xcp.tile([128, C], F32, tag="xo")
        nc.sync.dma_start(out=xo, in_=xres_d[qc * 128:(qc + 1) * 128, :])
        rst = xcp.tile([128, C], F16, tag="rst")
        nc.gpsimd.dma_start(out=rst, in_=rs_out[qc])
        nc.vector.tensor_add(x_mid[qc], rst, xo)
        st = stp2.tile([128, 2, 6], F32, tag="st2")
        xr = x_mid[qc].rearrange("p (g f) -> p g f", g=2)
        nc.vector.bn_stats(out=st[:, 0, :], in_=xr[:, 0, :])
        nc.vector.bn_stats(out=st[:, 1, :], in_=xr[:, 1, :])
        mv = stp2.tile([128, 2], F32, tag="mv2")
        nc.vector.bn_aggr(out=mv, in_=st)
        rstd = rstd_of(stp2, mv, "2")
        xc = xcp.tile([128, C], F16, tag="xc2")
        nc.vector.tensor_scalar(out=xc, in0=x_mid[qc], scalar1=mv[:, 0:1],
                                scalar2=rstd, op0=OP.subtract, op1=OP.mult)
        for cq in range(2):
            pt = paux.tile([128, 512], F16, tag="px", name="pt2")
            for i in range(4):
                cb = cq * 4 + i
                nc.tensor.matmul(
                    pt[:, 128 * i:128 * (i + 1)],
                    xc[:, cb * 128:(cb + 1) * 128], ident,
                    is_transpose=True, start=(i == 0), stop=(i == 3))
            nc.vector.tensor_copy(
                out=hlnT[:, cq * 4:cq * 4 + 4, qc * 128:(qc + 1) * 128],
                in_=pt.rearrange("p (i f) -> p i f", f=128))
        # x_mid becomes the final-residual base: fold in mlp-proj bias now
        nc.vector.tensor_add(x_mid[qc], x_mid[qc], bmp_bc)

    def fc_pass(t0, t1, h2gT, psm):
        # h2^T = gelu(wfc^T @ h_ln^T + b_fc) for row strips [t0, t1)
        n0, n1 = t0 * 128, t1 * 128
        for mg in range(4):
            for mt in range(8):
                m = mg * 8 + mt
                pf = psm.tile([128, 512], F32, tag="mm", name="pf")
                for k in range(CB):
                    nc.tensor.matmul(
                        pf[:, 0:n1 - n0],
                        wfc_sb[mg * 8 + k][:, mt * 128:(mt + 1) * 128],
                        hlnT[:, k, n0:n1], start=(k == 0),
                        stop=(k == CB - 1))
                nc.scalar.activation(
                    out=h2gT[:, m, n0:n1], in_=pf[:, 0:n1 - n0],
                    func=getattr(AF, GELU_NAME),
                    bias=bfc_sb[:, m:m + 1], scale=1.0)

    def g_pass(tlist, h2gT, psm, wmpp, outp):
        # out rows = h2g^T.T @ wmp + x_mid for the given (two) strips.
        # One [128,1024] wm DMA serves both cc halves: halves the trigger
        # count so the Sync queue can keep the weight stream ahead of PE.
        pg = {(t, cc): psm.tile([128, 512], F32, tag="mm",
                                name=f"pg{t}_{cc}")
              for t in tlist for cc in range(2)}
        for k in range(32):
            wm = wmpp.tile([128, 1024], F16, tag="wmp")
            nc.sync.dma_start(out=wm, in_=wmp_d[k * 128:(k + 1) * 128, :])
            for cc in range(2):
                for t in tlist:
                    nc.tensor.matmul(
                        pg[(t, cc)], h2gT[:, k, t * 128:(t + 1) * 128],
                        wm[:, cc * 512:(cc + 1) * 512],
                        start=(k == 0), stop=(k == 31))
        for t in tlist:
            for cc in range(2):
                ot = outp.tile([128, 512], F32, tag="ot")
                nc.vector.tensor_add(ot, pg[(t, cc)],
                                     x_mid[t][:, cc * 512:(cc + 1) * 512])
                nc.sync.dma_start(
                    out=out_d[t * 128:(t + 1) * 128,
                              cc * 512:(cc + 1) * 512],
                    in_=ot)

    # ================= program =================
    # x16 tile 15 is the very first DMA in the queue; weights go through
    # the (idle) GpSimd queue so the Sync queue serves activations. NOTE:
    # Tile dependency tracking is program-order, so every tile WRITE must
    # be emitted before any instruction that reads it — only the x DMA may
    # jump the queue, not the compute that consumes the weights.
    xts = {}
    for tt in (15, 14, 13, 12):
        xts[tt] = xpool.tile([128, C], F16, tag="xt", name=f"xt{tt}")
        nc.sync.dma_start(out=xts[tt], in_=x16_d[tt * 128:(tt + 1) * 128, :])
    for k in range(CB):
        nc.sync.dma_start(out=wv_sb[k], in_=wv_d[k * 128:(k + 1) * 128, :])
        nc.sync.dma_start(out=wqk_sb[k],
                          in_=wqk_d[k * 128:(k + 1) * 128, :])
    feed_tile(15, xt=xts[15])

    # feed descending; chunk-3 heads 0,1 interleave once their Q is ready
    A3_INTERLEAVE = True
    a3 = None
    for tt in range(TT - 2, -1, -1):
        feed_tile(tt, xt=xts.get(tt))
        if not A3_INTERLEAVE:
            continue
        if tt == 12:
            a3 = Attn3Interleaved()
        elif a3 is not None and tt < 12:
            a3.step(tt + 1)  # kb lags the feed by one tile (DVE slack)
    if A3_INTERLEAVE:
        for kb in (0, 13, 14, 15):
            a3.step(kb)
        yp3 = a3.finish()
    else:
        yp3 = [None, None]

    stp.release()
    xpool.release()
    xTp.release()
    pAB.release()
    psf.release()
    sreg = tc.alloc_tile_pool(name="sreg", bufs=2, space="PSUM")

    # right stack: resident wfc (loads overlap attention)
    wfcp = tc.alloc_tile_pool(name="wfcp", bufs=1, side="right")
    wfc_sb = [wfcp.tile([128, 1024], F16, tag=f"wfc{i}", name=f"wfc{i}")
              for i in range(32)]
    with tc.tile_wait_until(0.040):
        for mg in range(4):
            for k in range(CB):
                nc.scalar.dma_start(
                    out=wfc_sb[mg * 8 + k],
                    in_=wfc_d[k * 128:(k + 1) * 128,
                              mg * 1024:(mg + 1) * 1024])
    with tc.tile_wait_until(0.030):
        for k in range(2):
            nc.scalar.dma_start(out=wproj_sb[k],
                                in_=wproj_d[k * 128:(k + 1) * 128, :])
        nc.scalar.dma_start(out=bmp_bc, in_=_bc(bmp_d, 128))

    attention(3, ([2, 3] if A3_INTERLEAVE else [0, 1, 2, 3]), yp3, sreg)
    flush_all()
    proj_rs(3, yp3)
    yp2 = [None, None]
    attention(2, [0, 1, 2, 3], yp2, sreg)
    flush_all()
    proj_rs(2, yp2)
    resid_ln2(3, 0.100)
    yp1 = [None, None]
    attention(1, [0, 1, 2, 3], yp1, sreg)
    flush_all()
    proj_rs(1, yp1)
    resid_ln2(2, 0.115)
    yp0 = [None, None]
    attention(0, [0, 1, 2, 3], yp0, sreg)
    flush_all()
    proj_rs(0, yp0)
    resid_ln2(1, 0.130)
    probs.release()
    pBC.release()
    sreg.release()
    pyp.release()
    psm = tc.alloc_tile_pool(name="psm", bufs=7, space="PSUM")
    pFG = tc.alloc_tile_pool(name="pFG", bufs=1, side="right")
    wmpp = tc.alloc_tile_pool(name="wmpp", bufs=12, side="right")
    outp = tc.alloc_tile_pool(name="outp", bufs=3, side="right")
    h2gT = pFG.tile([128, 32, ROWS], F16, name="h2gT")
    with tc.tile_wait_until(0.115):
        fc_pass(2, 4, h2gT, psm)
    resid_ln2(0, 0.145)
    with tc.tile_wait_until(0.130):
        g_pass([3, 2], h2gT, psm, wmpp, outp)
    with tc.tile_wait_until(0.150):
        fc_pass(0, 2, h2gT, psm)
    with tc.tile_wait_until(0.165):
        g_pass([1, 0], h2gT, psm, wmpp, outp)

    outp.release()
    wmpp.release()
    pFG.release()
    wfcp.release()
    psm.release()
    prst.release()
    dsbp.release()
    ystg.release()
    wprojp.release()
    xcp.release()
    stp2.release()
    pEF.release()
    pEG.release()
    pyp_release = None  # (released above, before psm alloc)
    paux.release()
    dram.release()
    consts.release()


_CACHED = None


def _get_program():
    global _CACHED
    if _CACHED is None:
        _CACHED = build_program()
    return _CACHED


def _prep_inputs(inputs):
    """Fold LN params into weights and build the 8 per-core input maps."""
    x = np.asarray(inputs["x"], np.float32)
    ln1_w = np.asarray(inputs["ln1_w"], np.float32)
    ln1_b = np.asarray(inputs["ln1_b"], np.float32)
    w_attn = np.asarray(inputs["w_attn"], np.float32)
    b_attn = np.asarray(inputs["b_attn"], np.float32)
    w_proj = np.asarray(inputs["w_proj"], np.float32)
    b_proj = np.asarray(inputs["b_proj"], np.float32)
    ln2_w = np.asarray(inputs["ln2_w"], np.float32)
    ln2_b = np.asarray(inputs["ln2_b"], np.float32)
    w_fc = np.asarray(inputs["w_fc"], np.float32)
    b_fc = np.asarray(inputs["b_fc"], np.float32)
    w_mp = np.asarray(inputs["w_mlp_proj"], np.float32)
    b_mp = np.asarray(inputs["b_mlp_proj"], np.float32)

    Wa = ln1_w[:, None] * w_attn                      # [C, 3C]
    Ba = b_attn + ln1_b @ w_attn                      # [3C]
    s = 1.0 / np.sqrt(D)
    Wq = Wa[:, 0:C] * s
    Bq = Ba[0:C] * s
    Wk = Wa[:, C:2 * C]
    Bk = Ba[C:2 * C]
    Wv = Wa[:, 2 * C:3 * C]
    Bv = Ba[2 * C:3 * C]
    bproj_eff = (b_proj + Bv @ w_proj).astype(np.float32)

    Wfc = (ln2_w[:, None] * w_fc).astype(np.float32)
    Bfc = (b_fc + ln2_b @ w_fc).astype(np.float32)

    ident = np.eye(128, dtype=np.float16)
    trim = (np.arange(128)[:, None] <= np.arange(128)[None, :]).astype(
        np.float16)

    in_maps = []
    for c in range(N_CORES):
        g, p = divmod(c, TP)
        hs = slice(HPC * D * p, HPC * D * (p + 1))    # 256 cols/rows per core
        wqk = np.ascontiguousarray(
            np.concatenate([Wq[:, hs], Wk[:, hs]], axis=1), np.float16)
        bqk = np.ascontiguousarray(
            np.concatenate([Bq[hs], Bk[hs]]), np.float32)
        xres = np.concatenate(
            [x[g][512 * j + 128 * p:512 * j + 128 * p + 128]
             for j in range(QC)], axis=0) + bproj_eff[None, :]
        in_maps.append({
            "x16": np.ascontiguousarray(x[g]).astype(np.float16),
            "xres": np.ascontiguousarray(xres.astype(np.float32)),
            "wqk": wqk,
            "bqk": bqk,
            "wv": np.ascontiguousarray(Wv[:, hs]).astype(np.float16),
            "wproj": np.ascontiguousarray(w_proj[hs, :]).astype(np.float16),
            "wfc": Wfc.astype(np.float16),
            "bfc": Bfc,
            "wmp": w_mp.astype(np.float16),
            "bmp": b_mp,
            "ident": ident,
            "trim": trim,
        })
    return in_maps


def _gather(results):
    out = np.empty((B, T, C), np.float32)
    for c in range(N_CORES):
        g, p = divmod(c, TP)
        for j in range(QC):
            out[g, 512 * j + 128 * p:512 * j + 128 * p + 128, :] = \
                results[c]["out"][128 * j:128 * (j + 1)]
    return out


def kernel(**inputs) -> np.ndarray:
    nc = _get_program()
    in_maps = _prep_inputs(inputs)
    res = run_bass_kernel_spmd(nc, in_maps, list(range(N_CORES)))
    return _gather(res.results)


if __name__ == "__main__":
    print("building program...")
    _get_program()
    print("built ok")
